# revision 6
# baseline (speedup 1.0000x reference)
"""Distributed Trainium2 Bass kernel for 12-head attention with QK-RMSNorm and
2-axis interleaved RoPE (b=2, n=4096, dim=768, heads=12, head_dim=64).

Sharding: 8 cores = 2 (batch) x 4 (head groups of 3). Each core computes its 3
heads end-to-end (qkv proj, norm, rope, attention, out-proj partial) and the
host sums the 4 partial projections per batch.
"""
import sys, os
import numpy as np

for p in ("/opt/trn_rl_repo", "/root/.axon_site/_ro/trn_rl_repo"):
    if os.path.isdir(p) and p not in sys.path:
        sys.path.insert(0, p)

import concourse.bass as bass
import concourse.mybir as mybir
import concourse.tile as tile
from concourse.vector_clock import ScopedClock

F32 = mybir.dt.float32
BF16 = mybir.dt.bfloat16
I16 = mybir.dt.int16
AF = mybir.ActivationFunctionType
ALU = mybir.AluOpType

DIM, HEADS, HD = 768, 12, 64
N = 4096
NB = 512            # token block width
PI = float(np.pi)
TWO_PI = float(2 * np.pi)
INV2PI = float(1.0 / (2 * np.pi))
MAGIC = float(1.5 * 2 ** 23)
SINSCALE = 0.99998
LN8 = float(np.log(8.0))
# Schraudolph fast-exp into bf16 bit pattern via int16 convert (round-to-nearest)
EXP_A = float(2 ** 7 / np.log(2.0))
EXP_B = float(127 * 2 ** 7 - 7.4)
K_DVE = 8           # Bresenham-interleaved: (idx*K_DVE)%16 < K_DVE -> DVE fast-exp


# ---------------------------------------------------------------- tile patch
def _patched_drain_and_barrier(self, tick_clock, wait_clock):
    probe = self.nc.sync.nop()
    wait_clock.add_sem_waits(probe.ins, ScopedClock({None: tick_clock.global_clock}))
    si = probe.ins.sync_info
    waits = list(si.on_wait) if si is not None else []
    if len(waits) > 1:
        probe.ins.sync_info = mybir.SyncInfo(on_wait=[waits[0]], on_update=[])
        for w in waits[1:]:
            nxt = self.nc.sync.nop()
            nxt.ins.sync_info = mybir.SyncInfo(on_wait=[w], on_update=[])
    self.nc.sync.drain()
    self.nc.all_engine_barrier()
    assert self.sems is not None
    popped = self.nc._tile_sem_poison_stack.pop()
    assert popped is self._sem_poison
    self.nc.clear_and_free_semaphores(list(self.sems.allocated().values()))
    self.nc.all_engine_barrier()


tile.TileContext._drain_and_barrier = _patched_drain_and_barrier


def split_excess_waits(nc, max_waits=1):
    """Walrus only accepts one semaphore wait per instruction; move extras
    onto same-engine NOPs inserted before the instruction."""
    n_split = [0]

    def mk_nop(engine, waits):
        n_split[0] += 1
        nop = mybir.InstNoOp(name=f"waitsplit-{n_split[0]}", ins=[], outs=[])
        nop.engine = engine
        nop.sync_info = mybir.SyncInfo(on_wait=list(waits), on_update=[])
        return nop

    for f in nc.m.functions:
        for blk in f.blocks:
            out = []
            changed = False
            for inst in blk.instructions:
                si = inst.sync_info
                waits = list(si.on_wait) if si is not None else []
                if len(waits) > max_waits:
                    changed = True
                    extra, keep = waits[:-max_waits], waits[-max_waits:]
                    for i in range(0, len(extra), max_waits):
                        out.append(mk_nop(inst.engine, extra[i:i + max_waits]))
                    inst.sync_info = mybir.SyncInfo(
                        on_wait=keep, on_update=list(si.on_update))
                out.append(inst)
            if changed:
                blk.instructions = out
    return n_split[0]


def _reg_const(nc, dtype, val):
    t = nc.alloc_sbuf_tensor(f"const-{dtype.name}-{val}", [128, 1], dtype)
    nc.gpsimd.memset(t.ap(), val)
    nc.const_aps.aps[(dtype, val)] = t.ap()


# ---------------------------------------------------------------- builder
def build_nc():
    nc = bass.Bass("TRN2", target_bir_lowering=False, debug=False, num_devices=1)
    d = {}
    def din(name, shape, dt=BF16):
        d[name] = nc.declare_dram_parameter(name, shape, dt, isOutput=False)
    din("xT", [128, 6 * N])              # x[b].T, k-chunk-major, bf16
    din("wqq", [128, 6 * 128])           # [wq_h0.T | wq_h1.T] per k-chunk
    din("wkk", [128, 6 * 128])
    din("wqk2", [128, 6 * 128])          # [wq_h2.T | wk_h2.T]
    din("wv", [128, 6 * 192])            # [wv_h0.T | wv_h1.T | wv_h2.T]
    din("wp", [64, 3 * 768])             # proj slice rows per head
    din("cosT", [128, N])                # host-built rope tables (bf16)
    din("sinT", [128, N])
    din("sel", [128, 18])                # r2 column selectors (3 tiles x 6)
    din("bsel", [6, 3 * 128])            # rinv broadcast selectors per tile
    din("biasvec", [6, 1], F32)          # 0 for q rows, ln8 for k rows
    din("jmat", [128, 128])              # rotate-half pair swap matrix
    out_ext = nc.declare_dram_parameter("out", [N, DIM], BF16, isOutput=True)

    _reg_const(nc, F32, float(SINSCALE * PI / 2))
    _reg_const(nc, F32, 6.4e-5)
    nc.all_engine_barrier()

    with tile.TileContext(nc) as tc:
        with tc.tile_pool(name="persist", bufs=1) as pp:
            wqq = pp.tile([128, 6, 128], BF16)
            nc.sync.dma_start(wqq[:], d["wqq"][:].rearrange("p (c m) -> p c m", c=6))
            wkk = pp.tile([128, 6, 128], BF16)
            nc.sync.dma_start(wkk[:], d["wkk"][:].rearrange("p (c m) -> p c m", c=6))
            wqk2 = pp.tile([128, 6, 128], BF16)
            nc.sync.dma_start(wqk2[:], d["wqk2"][:].rearrange("p (c m) -> p c m", c=6))
            wv = pp.tile([128, 6, 192], BF16)
            nc.sync.dma_start(wv[:], d["wv"][:].rearrange("p (c m) -> p c m", c=6))
            wp = pp.tile([64, 3, 768], BF16)
            nc.sync.dma_start(wp[:], d["wp"][:].rearrange("p (h m) -> p h m", h=3))
            sel = pp.tile([128, 3, 6], BF16)
            nc.sync.dma_start(sel[:], d["sel"][:].rearrange("p (t s) -> p t s", t=3))
            bsel = pp.tile([6, 3, 128], BF16)
            nc.sync.dma_start(bsel[:], d["bsel"][:].rearrange("p (t m) -> p t m", t=3))
            bias6 = pp.tile([6, 1], F32)
            nc.sync.dma_start(bias6[:], d["biasvec"][:])
            jmat = pp.tile([128, 128], BF16)
            nc.sync.dma_start(jmat[:], d["jmat"][:])
            # broadcast helper: row at partition 64 -> psum rows 0:64 (T8 mm)
            bones = pp.tile([128, 128], BF16)
            nc.vector.memset(bones[:], 0.0)
            nc.gpsimd.memset(bones[64:65, 0:64], 1.0)
            # reciprocal denominators, zero-padded rows (3 head slots)
            rden_pad = pp.tile([128, 3, NB], BF16)
            nc.vector.memset(rden_pad[:], 0.0)
            # rope-normalized projections, [q_h0|q_h1], [k_h0|k_h1], [q_h2|k_h2]
            qq = pp.tile([128, N], BF16)
            kk = pp.tile([128, N], BF16)
            qk2 = pp.tile([128, N], BF16)
            # v with ones column per 128-token chunk: [128, kc, head, 65]
            v_sb = pp.tile([128, 32, 3, 65], BF16)
            nc.vector.memset(v_sb[:], 1.0)

            # ---------------- phase P: tables + projections + norm + rope
            with tc.tile_pool(name="phasep", bufs=2) as fp, \
                 tc.tile_pool(name="pp_qk", bufs=1, space="PSUM") as ps_qk, \
                 tc.tile_pool(name="pp_sm", bufs=1, space="PSUM") as ps_sm, \
                 tc.tile_pool(name="pp_rb", bufs=1, space="PSUM") as ps_rb:
                xT = fp.tile([128, 6, N], BF16, bufs=1)
                cosI = fp.tile([128, N], BF16, bufs=1)
                sinI = fp.tile([128, N], BF16, bufs=1)
                nc.sync.dma_start(cosI[:], d["cosT"][:])
                nc.sync.dma_start(sinI[:], d["sinT"][:])
                # split x load per block so compute starts early; DRAM is
                # nb-major so each block is one contiguous 6KB/partition read
                xTd = d["xT"][:].rearrange("p (b c m) -> p b c m", b=8, c=6)
                for nb in range(8):
                    sl = slice(nb * NB, (nb + 1) * NB)
                    nc.sync.dma_start(xT[:, :, sl], xTd[:, nb, :, :])

                tiles_w = [wqq, wkk, wqk2]
                for nb in range(8):
                    sl = slice(nb * NB, (nb + 1) * NB)
                    rots = []
                    r2 = ps_sm.tile([6, NB], F32, tag="r2", bufs=1)
                    for t in range(3):
                        qk_ps = ps_qk.tile([128, NB], F32, tag="qkp", bufs=3)
                        for kc in range(6):
                            nc.tensor.matmul(qk_ps[:], tiles_w[t][:, kc, :],
                                             xT[:, kc, sl], start=(kc == 0), stop=(kc == 5))
                        raw = fp.tile([128, NB], BF16, tag=f"raw{t}")
                        nc.scalar.copy(raw[:], qk_ps[:])
                        sq = fp.tile([128, NB], BF16, tag=f"sq{t}")
                        nc.gpsimd.tensor_tensor(sq[:], raw[:], raw[:], ALU.mult)
                        nc.tensor.matmul(r2[:], sel[:, t, :], sq[:],
                                         start=(t == 0), stop=(t == 2))
                        jp = ps_qk.tile([128, NB], F32, tag="jp", bufs=2)
                        nc.tensor.matmul(jp[:], jmat[:], raw[:], start=True, stop=True)
                        # rope rotation, off the post-norm critical path
                        t1 = fp.tile([128, NB], BF16, tag=f"t1_{t}")
                        nc.gpsimd.tensor_tensor(t1[:], raw[:], cosI[:, sl], ALU.mult)
                        t2 = fp.tile([128, NB], BF16, tag=f"t2_{t}")
                        nc.vector.tensor_tensor(t2[:], jp[:], sinI[:, sl], ALU.mult)
                        rot = fp.tile([128, NB], BF16, tag=f"rot{t}")
                        nc.vector.tensor_tensor(rot[:], t1[:], t2[:], ALU.add)
                        rots.append(rot)
                    lnr = fp.tile([6, NB], F32, tag="lnr")
                    nc.scalar.activation(lnr[:], r2[:], AF.Ln, bias=6.4e-5)
                    rinv = fp.tile([6, NB], BF16, tag="rinv")
                    nc.scalar.activation(rinv[:], lnr[:], AF.Exp, scale=-0.5, bias=bias6[:])
                    outs_t = [qq, kk, qk2]
                    for t in range(3):
                        rb = ps_rb.tile([128, NB], F32, tag="rb", bufs=1)
                        nc.tensor.matmul(rb[:], bsel[:, t, :], rinv[:], start=True, stop=True)
                        nc.vector.tensor_tensor(outs_t[t][:, sl], rots[t][:], rb[:], ALU.mult)
                    # v projection for the 4 chunks of this block
                    for s4 in range(4):
                        kcn = nb * 4 + s4
                        nsl = slice(nb * NB + s4 * 128, nb * NB + s4 * 128 + 128)
                        v_ps = ps_rb.tile([128, 192], F32, tag="vp", bufs=1)
                        for kc in range(6):
                            nc.tensor.matmul(v_ps[:], xT[:, kc, nsl], wv[:, kc, :],
                                             start=(kc == 0), stop=(kc == 5))
                        nc.scalar.copy(v_sb[:, kcn, :, 0:64], v_ps[:].rearrange(
                            "p (h e) -> p h e", h=3))

            # ---------------- phase A: attention + normalize + out-proj
            with tc.tile_pool(name="phasea", bufs=2) as fa, \
                 tc.tile_pool(name="pa_sp", bufs=1, space="PSUM") as ps_sp, \
                 tc.tile_pool(name="pa_acc", bufs=1, space="PSUM") as ps_acc:
                q2hi = fa.tile([128, N], BF16, bufs=1)
                k2lo = fa.tile([64, N], BF16, bufs=1)
                for nb in range(8):
                    sl = slice(nb * NB, (nb + 1) * NB)
                    nc.sync.dma_start(q2hi[64:128, sl], qk2[0:64, sl])
                    nc.sync.dma_start(k2lo[:, sl], qk2[64:128, sl])
                o_sb = fa.tile([64, 3, N], BF16, bufs=1)

                def emit_exp(pt, sp, idx):
                    if (idx * K_DVE) % 16 < K_DVE:
                        nc.vector.tensor_scalar(pt[:].bitcast(I16), sp[:],
                                                EXP_A, EXP_B, ALU.mult, ALU.add)
                    else:
                        nc.scalar.activation(pt[:], sp[:], AF.Exp)

                for qnb in range(8):
                    qsl = slice(qnb * NB, (qnb + 1) * NB)
                    oT = [ps_acc.tile([65, NB], F32, tag=f"oT{h}", bufs=1,
                                      name=f"oT{h}_{qnb}") for h in range(3)]
                    # heads 0/1 cross-packed on PE row groups; kc pairs batched
                    # so same-tile-mode matmuls stay adjacent in the PE stream
                    for kcp in range(16):
                        sps, pts = [], []
                        for kc in (2 * kcp, 2 * kcp + 1):
                            ksl = slice(kc * 128, (kc + 1) * 128)
                            sp = ps_sp.tile([128, 2 * NB], F32, tag="sp", bufs=2)
                            nc.tensor.matmul(sp[:, 0:NB], kk[0:64, ksl], qq[0:64, qsl],
                                             start=True, stop=True)
                            nc.tensor.matmul(sp[:, NB:2 * NB], kk[64:128, ksl],
                                             qq[64:128, qsl], start=True, stop=True)
                            sps.append(sp)
                        for i, kc in enumerate((2 * kcp, 2 * kcp + 1)):
                            pt = fa.tile([128, 2 * NB], BF16, tag="pt", bufs=6)
                            emit_exp(pt, sps[i], kc)
                            pts.append(pt)
                        for i, kc in enumerate((2 * kcp, 2 * kcp + 1)):
                            nc.tensor.matmul(oT[0][:], v_sb[:, kc, 0, :], pts[i][:, 0:NB],
                                             start=(kc == 0), stop=(kc == 31))
                            nc.tensor.matmul(oT[1][:], v_sb[:, kc, 1, :], pts[i][:, NB:2 * NB],
                                             start=(kc == 0), stop=(kc == 31))
                    # head 2 self-packed pairs
                    for kp in range(16):
                        kc0, kc1 = 2 * kp, 2 * kp + 1
                        sp = ps_sp.tile([128, 2 * NB], F32, tag="sp", bufs=2)
                        nc.tensor.matmul(sp[:, 0:NB], k2lo[:, kc0 * 128:(kc0 + 1) * 128],
                                         qk2[0:64, qsl], start=True, stop=True)
                        nc.tensor.matmul(sp[:, NB:2 * NB],
                                         qk2[64:128, kc1 * 128:(kc1 + 1) * 128],
                                         q2hi[64:128, qsl], start=True, stop=True)
                        pt = fa.tile([128, 2 * NB], BF16, tag="pt", bufs=6)
                        emit_exp(pt, sp, kp)
                        nc.tensor.matmul(oT[2][:], v_sb[:, kc0, 2, :], pt[:, 0:NB],
                                         start=(kc0 == 0), stop=False)
                        nc.tensor.matmul(oT[2][:], v_sb[:, kc1, 2, :], pt[:, NB:2 * NB],
                                         start=False, stop=(kc1 == 31))
                    # normalize: evacuate oT to SBUF (frees the PSUM bank early),
                    # reciprocal of the ones-column denominator, broadcast via a
                    # T8 matmul reading the zero-padded rden rows, then scale.
                    for h in range(3):
                        oc = fa.tile([65, NB], BF16, tag="oc", bufs=3,
                                     name=f"oc_{qnb}_{h}")
                        nc.vector.tensor_copy(oc[:], oT[h][:])
                        lnd = fa.tile([65, NB], F32, tag="lnd", bufs=2,
                                      name=f"lnd_{qnb}_{h}")
                        nc.scalar.activation(lnd[64:65, :], oc[64:65, :], AF.Ln)
                        nc.scalar.activation(rden_pad[64:65, h, :],
                                             lnd[64:65, :], AF.Exp, scale=-1.0)
                        rbp = ps_acc.tile([128, NB], F32, tag="op", bufs=1,
                                          name=f"rbp_{qnb}_{h}")
                        nc.tensor.matmul(rbp[:], bones[64:128, :], rden_pad[64:128, h, :],
                                         start=True, stop=True, tile_position=(64, 0))
                        rbo = fa.tile([64, NB], BF16, tag="rbo")
                        if h % 2 == 0:
                            nc.scalar.copy(rbo[:], rbp[0:64, :])
                        else:
                            nc.vector.tensor_copy(rbo[:], rbp[0:64, :])
                        nc.vector.tensor_tensor(o_sb[:, h, qsl], oc[0:64, :],
                                                rbo[:], ALU.mult)
                    # out-projection for this block's 4 row tiles
                    for s4 in range(4):
                        r0 = qnb * NB + s4 * 128
                        for mb in range(2):
                            msl = slice(mb * 384, (mb + 1) * 384)
                            op = ps_acc.tile([128, 384], F32, tag="op", bufs=1)
                            for h in range(3):
                                nc.tensor.matmul(op[:], o_sb[:, h, r0:r0 + 128],
                                                 wp[:, h, msl], start=(h == 0), stop=(h == 2))
                            opc = fa.tile([128, 384], BF16, tag="opc", bufs=3)
                            if (s4 + mb) % 2 == 0:
                                nc.vector.tensor_copy(opc[:], op[:])
                            else:
                                nc.scalar.copy(opc[:], op[:])
                            nc.sync.dma_start(out_ext[r0:r0 + 128, msl], opc[:])
    return nc


# ---------------------------------------------------------------- host prep
def _prep_core(inputs, core):
    import ml_dtypes
    bf = ml_dtypes.bfloat16
    NBH = 512
    b, g = core // 4, core % 4
    h0 = 3 * g
    x = np.asarray(inputs["x"])[b]                      # [N, 768]
    q_w = np.asarray(inputs["q_w"]); kv_w = np.asarray(inputs["kv_w"])
    proj_w = np.asarray(inputs["proj_w"]); inv_freq = np.asarray(inputs["inv_freq"])
    pos = np.asarray(inputs["positions"])[b]            # [N, 2]

    def chunk_major(a):  # [768, M] f32 -> [128, 6*M] bf16
        M = a.shape[1]
        return np.ascontiguousarray(
            a.reshape(6, 128, M).transpose(1, 0, 2).reshape(128, 6 * M)).astype(bf)

    # [768, N] -> chunks [6, 128, 8, 512] -> nb-major [128, 8, 6, 512]
    xTf = x.T.astype(np.float32).reshape(6, 128, 8, NBH)
    xT = np.ascontiguousarray(xTf.transpose(1, 2, 0, 3)).reshape(128, 6 * 4096).astype(bf)
    wq = [q_w[(h0 + h) * 64:(h0 + h + 1) * 64].T for h in range(3)]
    wk = [kv_w[(h0 + h) * 64:(h0 + h + 1) * 64].T for h in range(3)]
    wvv = [kv_w[768 + (h0 + h) * 64:768 + (h0 + h + 1) * 64].T for h in range(3)]
    wqq = chunk_major(np.concatenate([wq[0], wq[1]], 1))
    wkk = chunk_major(np.concatenate([wk[0], wk[1]], 1))
    wqk2 = chunk_major(np.concatenate([wq[2], wk[2]], 1))
    wv = chunk_major(np.concatenate(wvv, 1))
    wp = np.concatenate(
        [proj_w[:, (h0 + h) * 64:(h0 + h + 1) * 64].T for h in range(3)], 1).astype(bf)

    # host-built rope tables: partition p of q/k rows uses axis/freq per layout
    pclip = np.maximum(pos.astype(np.float64), 0.0)      # [N, 2]
    fr = np.zeros((128,), np.float64)
    ax = np.zeros((128,), np.int64)
    for dd in range(64):
        axis, j = (0, dd // 2) if dd < 32 else (1, (dd - 32) // 2)
        fr[dd] = fr[64 + dd] = inv_freq[j]
        ax[dd] = ax[64 + dd] = axis
    ang = pclip[:, ax].T * fr[:, None]                   # [128, N]
    cosT = np.cos(ang).astype(bf)
    sinT = np.sin(ang).astype(bf)

    sel = np.zeros((128, 18), np.float32)
    # r2 rows: 0..2 = q_h0..q_h2, 3..5 = k_h0..k_h2 ; tiles qq, kk, qk2
    for (t, lo_row, hi_row) in ((0, 0, 1), (1, 3, 4), (2, 2, 5)):
        sel[0:64, 6 * t + lo_row] = 1.0
        sel[64:128, 6 * t + hi_row] = 1.0
    bsel = np.zeros((6, 3 * 128), np.float32)
    for (t, lo_row, hi_row) in ((0, 0, 1), (1, 3, 4), (2, 2, 5)):
        bsel[lo_row, 128 * t:128 * t + 64] = 1.0
        bsel[hi_row, 128 * t + 64:128 * t + 128] = 1.0
    biasvec = np.array([[0.0], [0.0], [0.0], [LN8], [LN8], [LN8]], np.float32)

    jm = np.zeros((128, 128), np.float32)
    for base in range(0, 128, 32):
        for j in range(16):
            jm[base + 2 * j + 1, base + 2 * j] = -1.0
            jm[base + 2 * j, base + 2 * j + 1] = 1.0

    return {"xT": xT, "wqq": wqq, "wkk": wkk, "wqk2": wqk2, "wv": wv,
            "wp": wp.astype(bf), "cosT": cosT, "sinT": sinT,
            "sel": sel.astype(bf), "bsel": bsel.astype(bf), "biasvec": biasvec,
            "jmat": jm.astype(bf)}


_CACHE = {}


def kernel(**inputs) -> np.ndarray:
    from concourse.bass_utils import run_bass_kernel_spmd
    if "nc" not in _CACHE:
        nc = build_nc()
        split_excess_waits(nc, max_waits=1)
        _CACHE["nc"] = nc
    nc = _CACHE["nc"]
    in_maps = [_prep_core(inputs, c) for c in range(8)]
    res = run_bass_kernel_spmd(nc, in_maps, core_ids=list(range(8)))
    b0 = sum(res.results[g]["out"].astype(np.float32) for g in range(4))
    b1 = sum(res.results[4 + g]["out"].astype(np.float32) for g in range(4))
    return np.stack([b0, b1]).astype(np.float32)
